# revision 2
# baseline (speedup 1.0000x reference)
"""MAGNO encoder kernel for 8 Trainium2 NeuronCores.

Strategy:
  - Edges are sorted by destination latent node on the host (index-only prep).
    Core c owns latents [512c, 512(c+1)) and receives exactly the edges whose
    dst falls in its range -> no cross-core reduction is needed at all; the
    final output is the concatenation of per-core [512, 256] blocks.
  - Within a core, edges are bucketed by dst>>7 (4 buckets of 128 latents) and
    by src>>15 (4 groups, so gather indices fit int16 for dma_gather).
  - Per-edge phys features are gathered on-device with dma_gather(transpose=
    True) from an HBM table [131072, 128] fp16 -> XT [6, n] feature-major.
  - Algebra: edge_in @ W1 = [f,p] @ Wfp + latent_pos[dst] @ W1c with
    Wfp = [W1[0:3]; W1[3:6]-W1[6:9]].  LAT1 = latent_pos @ W1c + b1 is
    precomputed per latent on device; the per-edge latent contribution is
    applied with a one-hot matmul (contraction over the 128 bucket slots).
  - W3 is applied AFTER aggregation: sum_e msg = (sum_e gelu(h2)) @ W3, so the
    third layer runs per latent (4096 rows) instead of per edge (1M rows).
  - Scatter = one-hot matmul accumulating into a persistent PSUM bank per
    bucket; mean = sum * (1/max(cnt,1)) with cnt host-derived from indices
    and reciprocal computed on device.
"""

import os
import numpy as np

import concourse.bass as bass
import concourse.mybir as mybir
import concourse.tile as tile
from concourse import bacc
from concourse.bass_utils import run_bass_kernel_spmd

P = 128
N_PHYS = 100000
N_LATENT = 4096
HID = 256
NCORES = 8
LPC = N_LATENT // NCORES          # latents per core = 512
NBKT = LPC // P                   # dst buckets per core = 4
SRCB = 32768                      # src bucket size (int16 indexable)
NSRCG = (N_PHYS + SRCB - 1) // SRCB  # = 4
SUP = 512                         # superchunk edge count

f32 = mybir.dt.float32
f16 = mybir.dt.float16
i16 = mybir.dt.int16

last_results = None  # set by kernel(); test harness reads exec_time_ns


def _build_program(seg_len, b1nz, b2nz, b3nz):
    """seg_len[b][g]: padded edge count (mult of 128) for bucket b, src grp g.
    Returns (nc, names of inputs)."""
    nchunks = sum(seg_len[b][g] for b in range(NBKT) for g in range(NSRCG)) // P
    ne = nchunks * P

    nc = bacc.Bacc("TRN2", target_bir_lowering=False)

    # ---- inputs ----
    physcat_d = nc.dram_tensor("physcat", [NSRCG * SRCB, P], f16, kind="ExternalInput")
    totw = sum(seg_len[b][g] // 16 for b in range(NBKT) for g in range(NSRCG))
    idxw_d = nc.dram_tensor("idxw", [P, max(totw, 1)], i16, kind="ExternalInput")
    dstrep_d = nc.dram_tensor("dstrep", [P, ne], f16, kind="ExternalInput")
    dstcol_d = nc.dram_tensor("dstcol", [P, nchunks], f32, kind="ExternalInput")
    cnt_d = nc.dram_tensor("cnt", [P, NBKT], f32, kind="ExternalInput")
    latposT_d = nc.dram_tensor("latposT", [3, LPC], f32, kind="ExternalInput")
    W1_d = nc.dram_tensor("W1", [9, HID], f32, kind="ExternalInput")
    W1c_d = nc.dram_tensor("W1c", [3, HID], f32, kind="ExternalInput")
    W2p_d = nc.dram_tensor("W2p", [P, 2 * HID], f16, kind="ExternalInput")
    W3p_d = nc.dram_tensor("W3p", [P, 2 * HID], f16, kind="ExternalInput")
    b1_d = nc.dram_tensor("b1r", [1, HID], f32, kind="ExternalInput")
    b2h_d = nc.dram_tensor("b2h", [1, HID], f16, kind="ExternalInput")
    b3_d = nc.dram_tensor("b3r", [1, HID], f32, kind="ExternalInput")
    ST_d = nc.dram_tensor("ST", [9, 8], f32, kind="ExternalInput")
    iotaf_d = nc.dram_tensor("iotaf", [P, P], f32, kind="ExternalInput")
    iotach_d = nc.dram_tensor("iotach", [P, 1], f32, kind="ExternalInput")
    ones1_d = nc.dram_tensor("ones1", [1, P], f32, kind="ExternalInput")
    ones1h_d = nc.dram_tensor("ones1h", [1, P], f16, kind="ExternalInput")
    ident_d = nc.dram_tensor("ident", [P, P], f32, kind="ExternalInput")
    out_d = nc.dram_tensor("out", [LPC, HID], f32, kind="ExternalOutput")

    GELU = (mybir.ActivationFunctionType.Tanh
            if os.environ.get("MAGNO_SIM_ACT") == "tanh"
            else mybir.ActivationFunctionType.Gelu_apprx_tanh)

    with tile.TileContext(nc) as tc:
        with tc.tile_pool(name="const", bufs=1) as cp, \
             tc.tile_pool(name="psG", bufs=1, space="PSUM") as psG:

            # ---- persistent SBUF constants ----
            def load(shape, dt, src_ap, tag):
                t = cp.tile(shape, dt, tag=tag)
                nc.default_dma_engine.dma_start(out=t[:], in_=src_ap)
                return t

            iotaf_t = load([P, P], f32, iotaf_d[:], "iotaf")
            iotach_t = load([P, 1], f32, iotach_d[:], "iotach")
            ones1_t = load([1, P], f32, ones1_d[:], "ones1")
            ident_t = load([P, P], f32, ident_d[:], "ident")
            W1_t = load([9, HID], f32, W1_d[:], "W1")
            W1c_t = load([3, HID], f32, W1c_d[:], "W1c")
            W2p_t = load([P, 2 * HID], f16, W2p_d[:], "W2p")
            W3p_t = load([P, 2 * HID], f16, W3p_d[:], "W3p")
            ST_t = load([9, 8], f32, ST_d[:], "ST")
            latposT_t = load([3, LPC], f32, latposT_d[:], "latposT")
            cnt_t = load([P, NBKT], f32, cnt_d[:], "cnt")
            dstcol_t = load([P, nchunks], f32, dstcol_d[:], "dstcol")
            if b1nz:
                b1_t = load([1, HID], f32, b1_d[:], "b1")
            if b2nz:
                b2h_t = load([1, HID], f16, b2h_d[:], "b2h")
                ones1h_t = load([1, P], f16, ones1h_d[:], "ones1h")
            if b3nz:
                b3_t = load([1, HID], f32, b3_d[:], "b3")

            # rcnt = 1 / max(cnt, 1)
            cntm_t = cp.tile([P, NBKT], f32, tag="cntm")
            nc.vector.tensor_scalar(out=cntm_t[:], in0=cnt_t[:], scalar1=1.0,
                                    scalar2=None, op0=mybir.AluOpType.max)
            rcnt_t = cp.tile([P, NBKT], f32, tag="rcnt")
            nc.vector.reciprocal(out=rcnt_t[:], in_=cntm_t[:])

            # persistent per-bucket PSUM accumulators [128, 256]
            G_ps = [psG.tile([P, HID], f32, tag=f"G{b}", name=f"G{b}")
                    for b in range(NBKT)]

            # ---- setup phase: W1fp, LAT1 (+ b3 broadcast) ----
            with tc.tile_pool(name="psS", bufs=1, space="PSUM") as psS:
                w1fp_ps = psS.tile([8, HID], f32, tag="w1fp")
                nc.tensor.matmul(out=w1fp_ps[:], lhsT=ST_t[:], rhs=W1_t[:],
                                 start=True, stop=True)
                W1fph_t = cp.tile([8, HID], f16, tag="W1fph")
                nc.vector.tensor_copy(out=W1fph_t[:], in_=w1fp_ps[:])

                LAT1h = []
                for b in range(NBKT):
                    l_ps = psS.tile([P, HID], f32, tag="lat1")
                    nc.tensor.matmul(out=l_ps[:],
                                     lhsT=latposT_t[:, b * P:(b + 1) * P],
                                     rhs=W1c_t[:],
                                     start=True, stop=not b1nz)
                    if b1nz:
                        nc.tensor.matmul(out=l_ps[:], lhsT=ones1_t[:],
                                         rhs=b1_t[:], start=False, stop=True)
                    lt = cp.tile([P, HID], f16, tag=f"LAT1_{b}", name=f"LAT1_{b}")
                    nc.vector.tensor_copy(out=lt[:], in_=l_ps[:])
                    LAT1h.append(lt)

                if b3nz:
                    b3_ps = psS.tile([P, HID], f32, tag="b3bc")
                    nc.tensor.matmul(out=b3_ps[:], lhsT=ones1_t[:], rhs=b3_t[:],
                                     start=True, stop=True)
                    b3bc_t = cp.tile([P, HID], f32, tag="b3bc")
                    nc.vector.tensor_copy(out=b3bc_t[:], in_=b3_ps[:])

            # ---- main loop ----
            with tc.tile_pool(name="work", bufs=2) as wp, \
                 tc.tile_pool(name="psW", bufs=1, space="PSUM") as psW, \
                 tc.tile_pool(name="psA", bufs=2, space="PSUM") as psA:

                bkt_nch = [sum(seg_len[b][g] for g in range(NSRCG)) // P
                           for b in range(NBKT)]
                jglobal = 0      # global 128-chunk counter (dstcol column)
                eglobal = 0      # global edge offset (dstrep column)
                idxoff = 0       # offset into idxw (int16 cols)
                for b in range(NBKT):
                    jb = 0       # chunk counter within bucket
                    for g in range(NSRCG):
                        L = seg_len[b][g]
                        if L == 0:
                            continue
                        for o in range(0, L, SUP):
                            n = min(SUP, L - o)
                            # gather XT [128, 1, n] fp16 for this superchunk
                            # (dma_gather >512 idxs fails on HW; cap at SUP)
                            idx_t = wp.tile([P, SUP // 16], i16, tag="idx")
                            nc.default_dma_engine.dma_start(
                                out=idx_t[:, :n // 16],
                                in_=idxw_d[:, idxoff:idxoff + n // 16])
                            xt_t = wp.tile([P, 1, SUP], f16, tag="xt")
                            nreg = nc.gpsimd.snap(n)
                            nc.gpsimd.dma_gather(
                                out_ap=xt_t[:, :, :n],
                                in_ap=physcat_d[g * SRCB:(g + 1) * SRCB, :],
                                idxs_ap=idx_t[:, :n // 16],
                                num_idxs=n, num_idxs_reg=nreg,
                                elem_size=P, transpose=True)
                            idxoff += n // 16
                            dr_t = wp.tile([P, SUP], f16, tag="dstrep")
                            nc.default_dma_engine.dma_start(
                                out=dr_t[:, :n],
                                in_=dstrep_d[:, eglobal:eglobal + n])
                            ohse_t = wp.tile([P, SUP], f16, tag="ohse")
                            nc.vector.tensor_scalar(
                                out=ohse_t[:, :n], in0=dr_t[:, :n],
                                scalar1=iotach_t[:, 0:1], scalar2=None,
                                op0=mybir.AluOpType.is_equal)

                            a1_t = wp.tile([P, 2, SUP], f16, tag="a1")
                            for m in range(2):
                                h1_ps = psW.tile([P, SUP], f32, tag=f"h1_{m}")
                                nc.tensor.matmul(
                                    out=h1_ps[:, :n],
                                    lhsT=W1fph_t[0:6, m * P:(m + 1) * P],
                                    rhs=xt_t[0:6, 0, 0:n],
                                    start=True, stop=False)
                                nc.tensor.matmul(
                                    out=h1_ps[:, :n],
                                    lhsT=LAT1h[b][:, m * P:(m + 1) * P],
                                    rhs=ohse_t[:, :n],
                                    start=False, stop=True)
                                nc.scalar.activation(
                                    out=a1_t[:, m, :n], in_=h1_ps[:, :n],
                                    func=GELU)

                            for s in range(0, n, P):
                                ohes_t = wp.tile([P, P], f16, tag="ohes")
                                nc.vector.tensor_scalar(
                                    out=ohes_t[:], in0=iotaf_t[:],
                                    scalar1=dstcol_t[:, jglobal:jglobal + 1],
                                    scalar2=None,
                                    op0=mybir.AluOpType.is_equal)

                                a2_ps = psA.tile([P, HID], f32, tag="a2")
                                nc.tensor.matmul(
                                    out=a2_ps[:],
                                    lhsT=a1_t[:, 0, s:s + P],
                                    rhs=W2p_t[:, 0:HID],
                                    start=True, stop=False)
                                nc.tensor.matmul(
                                    out=a2_ps[:],
                                    lhsT=a1_t[:, 1, s:s + P],
                                    rhs=W2p_t[:, HID:2 * HID],
                                    start=False, stop=not b2nz)
                                if b2nz:
                                    nc.tensor.matmul(
                                        out=a2_ps[:], lhsT=ones1h_t[:],
                                        rhs=b2h_t[:], start=False, stop=True)
                                a2h_t = wp.tile([P, HID], f16, tag="a2h")
                                nc.scalar.activation(out=a2h_t[:], in_=a2_ps[:],
                                                     func=GELU)

                                nc.tensor.matmul(
                                    out=G_ps[b][:],
                                    lhsT=ohes_t[:], rhs=a2h_t[:],
                                    start=(jb == 0), stop=(jb == bkt_nch[b] - 1),
                                    skip_group_check=True)
                                jb += 1
                                jglobal += 1
                                eglobal += P

            # ---- epilogue: O = (G * rcnt) @ W3 + b3 per bucket ----
            with tc.tile_pool(name="ep", bufs=2) as ep, \
                 tc.tile_pool(name="psE", bufs=2, space="PSUM") as psE:
                for b in range(NBKT):
                    gs_t = ep.tile([P, HID], f32, tag="gs")
                    nc.vector.tensor_scalar(
                        out=gs_t[:], in0=G_ps[b][:],
                        scalar1=rcnt_t[:, b:b + 1], scalar2=None,
                        op0=mybir.AluOpType.mult)
                    gth_t = ep.tile([P, 2, P], f16, tag="gth")
                    for k in range(2):
                        gt_ps = psE.tile([P, P], f32, tag="gt")
                        nc.tensor.transpose(out=gt_ps[:],
                                            in_=gs_t[:, k * P:(k + 1) * P],
                                            identity=ident_t[:])
                        nc.vector.tensor_copy(out=gth_t[:, k, :], in_=gt_ps[:])
                    o_ps = psE.tile([P, HID], f32, tag="o")
                    nc.tensor.matmul(out=o_ps[:], lhsT=gth_t[:, 0, :],
                                     rhs=W3p_t[:, 0:HID], start=True, stop=False)
                    nc.tensor.matmul(out=o_ps[:], lhsT=gth_t[:, 1, :],
                                     rhs=W3p_t[:, HID:2 * HID],
                                     start=False, stop=True)
                    o_t = ep.tile([P, HID], f32, tag="osb")
                    if b3nz:
                        nc.vector.tensor_tensor(out=o_t[:], in0=o_ps[:],
                                                in1=b3bc_t[:],
                                                op=mybir.AluOpType.add)
                    else:
                        nc.vector.tensor_copy(out=o_t[:], in_=o_ps[:])
                    nc.default_dma_engine.dma_start(
                        out=out_d[b * P:(b + 1) * P, :], in_=o_t[:])

    nc.finalize()
    return nc


def kernel(phys_feats, phys_pos, latent_pos, edge_src, edge_dst,
           W1, b1, W2, b2, W3, b3):
    global last_results
    phys_feats = np.asarray(phys_feats, dtype=np.float32)
    phys_pos = np.asarray(phys_pos, dtype=np.float32)
    latent_pos = np.asarray(latent_pos, dtype=np.float32)
    W1 = np.asarray(W1, dtype=np.float32)
    W2 = np.asarray(W2, dtype=np.float32)
    W3 = np.asarray(W3, dtype=np.float32)
    b1 = np.asarray(b1, dtype=np.float32)
    b2 = np.asarray(b2, dtype=np.float32)
    b3 = np.asarray(b3, dtype=np.float32)
    src_all = np.asarray(edge_src).reshape(-1).astype(np.int64)
    dst_all = np.asarray(edge_dst).reshape(-1).astype(np.int64)
    E = src_all.shape[0]

    # ---- host-side index prep (sharding): sort by dst, bucket, group ----
    order = np.argsort(dst_all, kind="stable")
    ssrc, sdst = src_all[order], dst_all[order]
    bounds = np.searchsorted(sdst, np.arange(0, N_LATENT + 1, LPC))

    per_core = []
    counts = np.zeros((NCORES, NBKT, NSRCG), dtype=np.int64)
    for c in range(NCORES):
        cs, cd = ssrc[bounds[c]:bounds[c + 1]], sdst[bounds[c]:bounds[c + 1]]
        dl = cd - c * LPC
        key = (dl >> 7) * NSRCG + (cs >> 15)
        o2 = np.argsort(key, kind="stable")
        cs, dl, key = cs[o2], dl[o2], key[o2]
        per_core.append((cs, dl))
        cnt = np.bincount(key, minlength=NBKT * NSRCG)
        counts[c] = cnt.reshape(NBKT, NSRCG)

    seg_len = [[0] * NSRCG for _ in range(NBKT)]
    for b in range(NBKT):
        for g in range(NSRCG):
            m = int(counts[:, b, g].max())
            seg_len[b][g] = ((m + P - 1) // P) * P
        if sum(seg_len[b]) == 0:
            seg_len[b][0] = P  # ensure every bucket has >=1 chunk

    nchunks = sum(seg_len[b][g] for b in range(NBKT) for g in range(NSRCG)) // P
    ne = nchunks * P
    totw = ne // 16

    # ---- per-core input arrays ----
    physcat = np.zeros((NSRCG * SRCB, P), dtype=np.float16)
    physcat[:N_PHYS, 0:3] = phys_feats
    physcat[:N_PHYS, 3:6] = phys_pos

    Svk = np.zeros((9, 8), dtype=np.float32)
    Svk[0:3, 0:3] = np.eye(3)
    Svk[3:6, 3:6] = np.eye(3)
    Svk[6:9, 3:6] = -np.eye(3)
    # W1fp rows: [W1[0:3]; W1[3:6]-W1[6:9]]  => lhsT = Svk s.t. Svk.T@W1 works
    # (Svk[k, m] so that sum_k Svk[k,m] W1[k,:] = row m)

    W2p = np.ascontiguousarray(
        W2.reshape(2, P, HID).transpose(1, 0, 2).reshape(P, 2 * HID)
    ).astype(np.float16)
    W3p = np.ascontiguousarray(
        W3.reshape(2, P, HID).transpose(1, 0, 2).reshape(P, 2 * HID)
    ).astype(np.float16)

    iotaf = np.tile(np.arange(P, dtype=np.float32), (P, 1))
    iotach = np.arange(P, dtype=np.float32)[:, None]
    ones1 = np.ones((1, P), dtype=np.float32)
    ones1h = np.ones((1, P), dtype=np.float16)
    ident = np.eye(P, dtype=np.float32)

    b1nz, b2nz, b3nz = bool(b1.any()), bool(b2.any()), bool(b3.any())

    in_maps = []
    for c in range(NCORES):
        cs, dl = per_core[c]
        key = (dl >> 7) * NSRCG + (cs >> 15)
        idxw = np.zeros((P, max(totw, 1)), dtype=np.int16)
        dst_pad = np.full(ne, -1.0, dtype=np.float32)
        io = 0
        eo = 0
        for b in range(NBKT):
            for g in range(NSRCG):
                L = seg_len[b][g]
                if L == 0:
                    continue
                sel = key == b * NSRCG + g
                segsrc = cs[sel] - g * SRCB
                segdst = dl[sel] - b * P
                nreal = segsrc.shape[0]
                idx = np.zeros(L, dtype=np.int16)
                idx[:nreal] = segsrc
                w = np.tile(idx.reshape(L // 16, 16).T, (NCORES, 1))
                idxw[:, io:io + L // 16] = w
                io += L // 16
                dst_pad[eo:eo + nreal] = segdst
                eo += L
        dstrep = np.broadcast_to(dst_pad.astype(np.float16), (P, ne))
        dstcol = np.ascontiguousarray(dst_pad.reshape(nchunks, P).T)
        cnt = np.zeros((P, NBKT), dtype=np.float32)
        dlc = np.bincount(dl, minlength=LPC).astype(np.float32)
        cnt[:, :] = dlc.reshape(NBKT, P).T
        latposT = np.ascontiguousarray(latent_pos[c * LPC:(c + 1) * LPC].T)

        in_maps.append(dict(
            physcat=physcat, idxw=idxw,
            dstrep=np.ascontiguousarray(dstrep), dstcol=dstcol, cnt=cnt,
            latposT=latposT, W1=W1, W1c=np.ascontiguousarray(W1[6:9]), W2p=W2p, W3p=W3p,
            b1r=b1[None, :], b2h=b2[None, :].astype(np.float16),
            b3r=b3[None, :], ST=Svk,
            iotaf=iotaf, iotach=iotach, ones1=ones1, ones1h=ones1h,
            ident=ident,
        ))

    nc = _build_program(seg_len, b1nz, b2nz, b3nz)
    global last_nc, last_in_maps
    last_nc, last_in_maps = nc, in_maps
    trace = bool(int(os.environ.get("MAGNO_TRACE", "0")))
    ncores_run = int(os.environ.get("MAGNO_CORES", str(NCORES)))
    res = run_bass_kernel_spmd(nc, in_maps[:ncores_run],
                               core_ids=list(range(ncores_run)), trace=trace)
    last_results = res
    return np.concatenate([res.results[c]["out"] for c in range(ncores_run)],
                          axis=0)

